# revision 30
# baseline (speedup 1.0000x reference)
"""Trainium2 Bass kernel for a dense transformer block.

Block: split-LayerNorm -> attention -> residual -> split-LayerNorm -> MLP(GELU)
-> residual.  Shapes: B=8, N=1024, D=768, H=12 heads (hd=64), HID=3072.

Sharding: pure data-parallel over batch -- one batch element per NeuronCore
(8 cores), all weights replicated, no collectives.

On-chip layout is feature-major (activations stored transposed, [feature,
token]).  The heavy GEMMs (qkv, attn*V, proj, fc1, fc2) run in fp8e4m3 with
MatmulPerfMode.DoubleRow (K=256 contraction per pass).  LayerNorm statistics
use float32r moving operands against tiny [128,3] indicator stationaries; the
per-token scale/offset (gamma*rstd, -gamma*mean*rstd + beta) are broadcast to
all 128 feature partitions with one [4,128]x[4,512] matmul each, so the apply
step is two plain tensor_tensor ops writing fp8 directly.  rstd is computed as
exp(-0.5*ln(var+eps)) so the scalar engine stays on the natural_log_exp table
set through LayerNorm and softmax (no sqrt-table swaps).  Attention scores use
K=64 partition-offset matmuls directly on the stacked q/k tiles (no zero-padded
key copies).
"""

import os
import numpy as np
import ml_dtypes

import concourse.bass as bass
import concourse.tile as tile
from concourse import bacc, mybir
from concourse.bass_utils import run_bass_kernel_spmd
from contextlib import ExitStack

F32 = mybir.dt.float32
F32R = mybir.dt.float32r
BF16 = mybir.dt.bfloat16
F8 = mybir.dt.float8e4
AF = mybir.ActivationFunctionType
ALU = mybir.AluOpType
DR = mybir.MatmulPerfMode.DoubleRow

D = 768
H = 12
HD = 64
HID = 3072
NT = 1024  # tokens per core
B = 8
S1 = 320  # split-LN segment boundaries: [0,320), [320,384), [384,768)
S2 = 384
SCALE = 0.125  # (D//H) ** -0.5 = 64 ** -0.5
EPS = 1e-5
P = 128

KC = D // P        # 6  c-chunks
KP = KC // 2       # 3  c-chunk pairs
MFC1 = HID // P    # 24
MP2 = MFC1 // 2    # 12 fc2 contraction pairs
VG = 80            # per-head col group in vT tiles: 64 dims + 1 ones + pad


def _halves():
    return (slice(0, 512), slice(512, 1024))


def _layernorm(tc, xat, sg4, bcols, indsum_s, invlen, eps_t, out_slab, tag,
               sq_scalar=False):
    """Split-LayerNorm over the feature dim (partitions), full token width.
    xat(k) returns the [128, NT] f32r feature-major input tile for chunk k.
    out_slab(k) returns the [128, NT] fp8 output AP for chunk k.
    sg4 is [4, D] f32r: rows 0..2 = gamma[p] masked by segment, row 3 =
    beta[p].  The scale/offset broadcast matmuls consume mv rows 0..2
    (rstd / -mean*rstd) plus a constant row 3 (0 / 1) so beta rides along."""
    nc = tc.nc
    with tc.tile_pool(name=f"lnsq_{tag}", bufs=3) as sqp, \
         tc.tile_pool(name=f"lnt_{tag}", bufs=3) as tp, \
         tc.tile_pool(name=f"lns_{tag}", bufs=1) as lns, \
         tc.tile_pool(name=f"lnps_{tag}", bufs=2, space="PSUM") as psstat, \
         tc.tile_pool(name=f"lnpb_{tag}", bufs=2, space="PSUM") as psb:
        sums = psstat.tile([3, NT], F32, tag="stat", name=f"sums_{tag}")
        sumsq = psstat.tile([3, NT], F32, tag="stat", name=f"sumsq_{tag}")
        xqs = [sqp.tile([P, NT], F32R, tag="xq", name=f"xq_{tag}_{k}")
               for k in range(KC)]
        for hi in range(2):
            hs = _halves()[hi]
            for k in range(KC):
                xk = xat(k)
                xf = xk[:, hs].bitcast(F32)
                if sq_scalar:
                    nc.scalar.activation(xqs[k][:, hs], xf, AF.Square)
                else:
                    nc.vector.tensor_tensor(xqs[k][:, hs], xf, xf, ALU.mult)
                nc.tensor.matmul(sums[:, hs], indsum_s[:, 3 * k:3 * k + 3],
                                 xk[:, hs],
                                 start=(k == 0), stop=(k == KC - 1))
                nc.tensor.matmul(sumsq[:, hs], indsum_s[:, 3 * k:3 * k + 3],
                                 xqs[k][:, hs],
                                 start=(k == 0), stop=(k == KC - 1))
        mean = lns.tile([3, NT], F32, tag="mean")
        nmsq = lns.tile([3, NT], F32, tag="nmsq")
        var = lns.tile([3, NT], F32, tag="var")
        std = lns.tile([3, NT], F32, tag="std")
        rcp = lns.tile([3, NT], F32, tag="rcp")
        # mv[:,0,:] = rstd ; mv[:,1,:] = -mean*rstd
        mv = lns.tile([3, 2, NT], F32R, tag="mv")
        nc.vector.tensor_scalar_mul(mean[:], sums[:], invlen[:])
        nc.vector.scalar_tensor_tensor(nmsq[:], mean[:], -1.0,
                                       mean[:], ALU.mult, ALU.mult)
        nc.vector.scalar_tensor_tensor(var[:], sumsq[:], invlen[:],
                                       nmsq[:], ALU.mult, ALU.add)
        nc.scalar.activation(std[:], var[:], AF.Sqrt, bias=eps_t[:])
        nc.vector.reciprocal_approx_fast(rcp[:], std[:])
        nc.vector.tensor_copy(mv[0:3, 0, :], rcp[:])
        nc.vector.scalar_tensor_tensor(mv[0:3, 1, :], mean[:], -1.0,
                                       rcp[:], ALU.mult, ALU.mult)
        for k in range(KC):
            sg_k = sg4[0:3, k * P:(k + 1) * P]
            scaleB = psb.tile([P, NT], F32, tag="bb")
            offB = psb.tile([P, NT], F32, tag="bb")
            for hs in _halves():
                nc.tensor.matmul(scaleB[:, hs], sg_k, mv[:, 0, hs],
                                 start=True, stop=True)
                nc.tensor.matmul(offB[:, hs], sg_k, mv[:, 1, hs],
                                 start=True, stop=True)
            tmp = tp.tile([P, NT], F32, tag="tmp", name=f"tmp_{tag}_{k}")
            nc.vector.tensor_tensor(tmp[:], xat(k).bitcast(F32), scaleB[:],
                                    ALU.mult)
            # out = (g*(-mean*rstd)_bcast + beta) + g*rstd*x
            nc.vector.scalar_tensor_tensor(out_slab(k), offB[:],
                                           bcols[:, k:k + 1], tmp[:],
                                           ALU.add, ALU.add)


def build():
    nc = bacc.Bacc("TRN2", target_bir_lowering=False, debug=False)

    xT = nc.dram_tensor("xT", [D, NT], F32R, kind="ExternalInput")
    wqk8d = nc.dram_tensor("wqk8", [KP * P, 2 * 2 * D], F8, kind="ExternalInput")
    wv8d = nc.dram_tensor("wv8", [KP * P, 2 * D], F8, kind="ExternalInput")
    wp8d = nc.dram_tensor("wp8", [KP * P, 2 * D], F8, kind="ExternalInput")
    wf18d = nc.dram_tensor("wf18", [KP * P, 2 * HID], F8, kind="ExternalInput")
    wf28d = nc.dram_tensor("wf28", [MP2 * P, 2 * D], F8, kind="ExternalInput")
    # constpack cols: 0:6 proj_b, 6:30 fc1_b, 30:36 fc2_b, 36:42 ln1_b,
    # 42:48 ln2_b, 48 invlen (rows 0..2)
    cpackd = nc.dram_tensor("cpack", [P, 49], F32, kind="ExternalInput")
    browd = nc.dram_tensor("brow", [1, 2 * D], BF16, kind="ExternalInput")
    sg1d = nc.dram_tensor("sg1", [4, D], F32R, kind="ExternalInput")
    sg2d = nc.dram_tensor("sg2", [4, D], F32R, kind="ExternalInput")
    indsd = nc.dram_tensor("indsum", [P, 3 * KC], F32R, kind="ExternalInput")
    outT = nc.dram_tensor("outT", [D, NT], F32, kind="ExternalOutput")

    with tile.TileContext(nc) as tc, ExitStack() as ctx:
        const = ctx.enter_context(tc.tile_pool(name="const", bufs=1))

        # constants
        eps_t = const.tile([3, 1], F32)
        nc.vector.memset(eps_t[:], EPS)
        # prewarm activation tables while the x DMA streams in.  Ln first so
        # the natural_log_exp set (which contains Exp too) is resident.
        warm = const.tile([1, 1], F32)
        nc.vector.memset(warm[:], 0.25)
        for fn in (AF.Exp, AF.Sqrt):
            wo = const.tile([1, 1], F32, tag=f"warm_{fn}")
            nc.scalar.activation(wo[:], warm[:], fn)

        # x first: LN1 start gates everything.  Column-split so the first
        # token-half lands before the second.
        x0pool = ctx.enter_context(tc.tile_pool(name="x0pool", bufs=KC))
        x_tiles = [x0pool.tile([P, NT], F32R, tag="x0", name=f"x0_{k}")
                   for k in range(KC)]
        for hs in _halves():
            for k in range(KC):
                nc.sync.dma_start(x_tiles[k][:, hs],
                                  xT[k * P:(k + 1) * P, hs])

        # small consts + attention weights on the scalar (Act) HWDGE queue so
        # they don't serialize behind x on the sync queue.
        indsum_s = const.tile([P, 3 * KC], F32R)
        nc.scalar.dma_start(indsum_s[:], indsd[:])
        sg1 = const.tile([4, D], F32R)
        nc.scalar.dma_start(sg1[:], sg1d[:])
        sg2 = const.tile([4, D], F32R)
        nc.scalar.dma_start(sg2[:], sg2d[:])
        cpack = const.tile([P, 49], F32)
        nc.scalar.dma_start(cpack[:], cpackd[:])
        pb = cpack[:, 0:6]
        f1b = cpack[:, 6:30]
        f2b = cpack[:, 30:36]
        b1c = cpack[:, 36:42]
        b2c = cpack[:, 42:48]
        invlen = cpack[0:3, 48:49]
        ones_row = const.tile([1, NT], BF16)
        nc.vector.memset(ones_row[:], 1.0)
        brow = const.tile([1, 2 * D], BF16)
        nc.scalar.dma_start(brow[:], browd[:])

        # proj weight pool first (outlives the attention-weight pool); its DMA
        # is issued after the attention weights so they win the bandwidth race.
        wlate = ctx.enter_context(ExitStack())
        wp_pool = wlate.enter_context(tc.tile_pool(name="wp_pool", bufs=1))
        wp8 = wp_pool.tile([P, KP, 2, D], F8, tag="wp8")
        # attention weights (single strided DMA each)
        wq_stage = ctx.enter_context(ExitStack())
        wq_pool = wq_stage.enter_context(tc.tile_pool(name="wqk", bufs=2))
        wqk8 = wq_pool.tile([P, KP, 2, 2 * D], F8, tag="wqk8")
        nc.scalar.dma_start(
            wqk8[:], wqk8d.ap().rearrange("(a p) (s x) -> p a s x", p=P, s=2))
        wv8 = wq_pool.tile([P, KP, 2, D], F8, tag="wv8")
        nc.scalar.dma_start(
            wv8[:], wv8d.ap().rearrange("(a p) (s x) -> p a s x", p=P, s=2))
        nc.sync.dma_start(
            wp8[:], wp8d.ap().rearrange("(a p) (s x) -> p a s x", p=P, s=2))

        # ---- LN1 + qkv + vT, pipelined per token-half ----
        v_stage = ctx.enter_context(ExitStack())   # vT tiles, live thru attention
        v_pool = v_stage.enter_context(tc.tile_pool(name="v_pool", bufs=4))
        nx_stage = ctx.enter_context(ExitStack())  # normx pairs, live thru qkv/vT
        nx_pool = nx_stage.enter_context(tc.tile_pool(name="nx_pool", bufs=KP))
        nxp = [nx_pool.tile([P, 2, NT], F8, tag="nxp", name=f"nxp{i}")
               for i in range(KP)]
        y_stage = ctx.enter_context(ExitStack())
        y_pool = y_stage.enter_context(
            tc.tile_pool(name="y_pool", bufs=KP, side="right"))
        ytp = [y_pool.tile([P, 2, NT], F8, tag="ytp", name=f"ytp{i}")
               for i in range(KP)]
        qkv_stage = ctx.enter_context(ExitStack())
        q_pool = qkv_stage.enter_context(
            tc.tile_pool(name="q_pool", bufs=12, side="right"))
        qt = [q_pool.tile([P, NT], BF16, tag="qkt", name=f"qt{m}")
              for m in range(2 * KC)]
        vtp = []
        for p2 in range(4):
            vt = v_pool.tile([P, 2, H * VG], F8, tag="vtp", name=f"vtp{p2}")
            # ones columns (col 64 of each 80-wide head group) for denominators
            for s in range(2):
                nc.vector.memset(
                    vt[:, s, :].rearrange("p (h c) -> p h c", c=VG)[:, :, HD:VG],
                    1.0)
            vtp.append(vt)

        # ---- LN1 both halves, then a dense qkv+vT block, then attention.
        # Keeping the dense matmul stream contiguous locks the PE clock gate
        # at full rate before the (scalar-bound) softmax stream starts. ----
        _layernorm(tc, lambda k: x_tiles[k][:], sg1, b1c, indsum_s, invlen,
                   eps_t, lambda k: nxp[k // 2][:, k % 2, :], "ln1")

        psqv_stage = ctx.enter_context(ExitStack())
        psqv = psqv_stage.enter_context(
            tc.tile_pool(name="psqv", bufs=4, space="PSUM"))

        # q/k chunks in head-consumption order; PSUM->SBUF copies alternate
        # between the scalar and vector engines so neither becomes the drain.
        QKV_ORDER = [0, 6, 1, 7, 2, 8, 3, 9, 4, 10, 5, 11]
        nu = 0
        for m in QKV_ORDER:
            for hi in range(2):
                hs = _halves()[hi]
                ps = psqv.tile([P, NT], F32, tag="qv", name=f"qv_{hi}_{m}")
                for k2 in range(KP):
                    nc.tensor.matmul(ps[:, 0:512],
                                     wqk8[:, k2, :, m * P:(m + 1) * P],
                                     nxp[k2][:, :, hs],
                                     start=(k2 == 0), stop=(k2 == KP - 1),
                                     perf_mode=DR)
                if nu % 2 == 0:
                    nc.scalar.activation(qt[m][:, hs], ps[:, 0:512], AF.Identity)
                else:
                    nc.vector.tensor_copy(qt[m][:, hs], ps[:, 0:512])
                nu += 1
        for tc_i in range(8):
            vps = psqv.tile([P, NT], F32, tag="qv", name=f"vps_{tc_i}")
            for cs in (slice(0, 512), slice(512, D)):
                for k2 in range(KP):
                    nc.tensor.matmul(vps[:, cs],
                                     nxp[k2][:, :, tc_i * P:(tc_i + 1) * P],
                                     wv8[:, k2, :, cs],
                                     start=(k2 == 0), stop=(k2 == KP - 1),
                                     perf_mode=DR)
            dst = vtp[tc_i // 2][:, tc_i % 2, :].rearrange(
                "p (h c) -> p h c", c=VG)[:, :, 0:HD]
            nc.vector.tensor_copy(
                dst, vps[:, 0:D].rearrange("p (h c) -> p h c", c=HD))
        # pre-load the exp table while the qkv block runs
        wexp = const.tile([1, 1], F32, tag="warm_exp2")
        nc.scalar.activation(wexp[:], warm[:], AF.Exp)
        psqv_stage.close()

        # ---- attention: scores/exp stream with 1-head lagged attn*V and
        # the late q/k chunks interleaved as sparse fillers ----
        psmm_stage = ctx.enter_context(ExitStack())
        psmm = psmm_stage.enter_context(
            tc.tile_pool(name="psmm", bufs=2, space="PSUM"))
        with tc.tile_pool(name="e_pool", bufs=10) as e_pool, \
             tc.tile_pool(name="kp_pool", bufs=4) as kp_pool, \
             tc.tile_pool(name="psot", bufs=2, space="PSUM") as psot, \
             tc.tile_pool(name="sm_pool", bufs=2) as sm_pool:

            # alternating zero-padded key tiles: the zero half is set once
            # and never overwritten (head parity matches buffer parity)
            kp_bufs = [kp_pool.tile([P, NT], BF16, tag="kp", name=f"kp{i}")
                       for i in range(4)]
            for i in range(4):
                if i % 2 == 0:
                    nc.vector.memset(kp_bufs[i][HD:P, :], 0.0)
                else:
                    nc.vector.memset(kp_bufs[i][0:HD, :], 0.0)

            def make_tail(h, ot):
                def tail():
                    dnm = sm_pool.tile([1, NT], F32, tag="dnm")
                    nc.vector.tensor_copy(dnm[:], ot[HD:HD + 1, :])
                    r = sm_pool.tile([1, NT], F32, tag="recip")
                    nc.vector.reciprocal_approx_fast(r[:], dnm[:])
                    rbs = sm_pool.tile([HD, NT], F32, tag="rbs")
                    nc.gpsimd.partition_broadcast(rbs[:], r[:])
                    po2 = (h % 2) * HD
                    nc.vector.tensor_mul(
                        ytp[h // 4][po2:po2 + HD, (h // 2) % 2, :],
                        ot[0:HD, :], rbs[:])
                return tail

            av_queue = []
            for h in range(H):
                j = h // 2
                po = (h % 2) * HD
                qsl = qt[j]
                ksl = qt[KC + j]
                kp = kp_bufs[h % 4]
                nc.vector.tensor_copy(kp[po:po + HD, :], ksl[po:po + HD, :])
                ot = psot.tile([P, NT], F32, tag="ot")
                exs = [e_pool.tile([P, 2, NT], F8, tag="ex", name=f"ex{h}_{i}")
                       for i in range(4)]

                def av(p2, ot=ot, exs=exs, hh=h, tl=None):
                    for hs in _halves():
                        nc.tensor.matmul(
                            ot[0:HD + 2, hs],
                            vtp[p2][:, :, hh * VG:hh * VG + HD + 2],
                            exs[p2][:, :, hs],
                            start=(p2 == 0), stop=(p2 == 3),
                            perf_mode=DR)
                    if tl is not None:
                        tl()

                for kc in range(8):
                    st = psmm.tile([P, NT], F32, tag="mm")
                    for hs in _halves():
                        nc.tensor.matmul(
                            st[:, hs],
                            kp[:, kc * P:(kc + 1) * P],
                            qsl[:, hs],
                            start=True, stop=True)
                    nc.scalar.activation(exs[kc // 2][:, kc % 2, :],
                                         st[:], AF.Exp, scale=SCALE / 256.0)
                    if kc % 2 == 1 and av_queue:
                        av_queue.pop(0)()
                tl = make_tail(h, ot)
                av_queue += [
                    lambda av=av: av(0), lambda av=av: av(1),
                    lambda av=av: av(2), lambda av=av, tl=tl: av(3, tl=tl)]
            while av_queue:
                av_queue.pop(0)()
        nx_stage.close()
        v_stage.close()
        wq_stage.close()
        qkv_stage.close()
        psmm_stage.close()

        # MLP weights: issue DMAs now (consumed ~20us later)
        wf1_pool = wlate.enter_context(tc.tile_pool(name="wf1_pool", bufs=1))
        wf18 = wf1_pool.tile([P, KP, 2, HID], F8, tag="wf18")
        nc.sync.dma_start(
            wf18[:], wf18d.ap().rearrange("(a p) (s x) -> p a s x", p=P, s=2))
        wf2_pool = wlate.enter_context(tc.tile_pool(name="wf2_pool", bufs=1))
        wf28 = wf2_pool.tile([P, MP2, 2, D], F8, tag="wf28")
        nc.sync.dma_start(
            wf28[:], wf28d.ap().rearrange("(a p) (s x) -> p a s x", p=P, s=2))
        x1pool = ctx.enter_context(tc.tile_pool(name="x1pool", bufs=KC))
        x1m = [x1pool.tile([P, NT], F32R, tag="x1", name=f"x1_{m}")
               for m in range(KC)]
        nx2_stage = ctx.enter_context(ExitStack())
        nx2_pool = nx2_stage.enter_context(tc.tile_pool(name="nx2_pool", bufs=KP))
        nx2p = [nx2_pool.tile([P, 2, NT], F8, tag="nx2p", name=f"nx2p{i}")
                for i in range(KP)]

        # ---- proj + residual (full token width), then LN2 ----
        with tc.tile_pool(name="pspj", bufs=2, space="PSUM") as pspj, \
             tc.tile_pool(name="ptp", bufs=2) as ptp:
            for m in range(KC):
                ps = pspj.tile([P, NT], F32, tag="pj")
                for hi, hs in enumerate(_halves()):
                    for k2 in range(KP):
                        nc.tensor.matmul(ps[:, hs],
                                         wp8[:, k2, :, m * P:(m + 1) * P],
                                         ytp[k2][:, :, hs],
                                         start=(k2 == 0), stop=False,
                                         perf_mode=DR)
                    # accumulate 256*proj_b via a K=1 rank-1 matmul
                    nc.tensor.matmul(ps[:, hs], brow[0:1, m * P:(m + 1) * P],
                                     ones_row[0:1, hs], start=False, stop=True)
                # x1 = (proj_acc)/256 + x
                nc.vector.scalar_tensor_tensor(
                    x1m[m][:], ps[:], 1.0 / 256.0,
                    x_tiles[m][:].bitcast(F32), ALU.mult, ALU.add)
        _layernorm(tc, lambda k: x1m[k][:], sg2, b2c, indsum_s, invlen,
                   eps_t, lambda k: nx2p[k // 2][:, k % 2, :], "ln2",
                   sq_scalar=True)
        y_stage.close()

        # ---- MLP, full token width, fc1 software-pipelined by 2 chunks ----
        h_stage = ctx.enter_context(ExitStack())
        h_pool = h_stage.enter_context(
            tc.tile_pool(name="h_pool", bufs=MP2, side="right"))
        h8 = [h_pool.tile([P, 2, NT], F8, tag="h8", name=f"h8_{i}")
              for i in range(MP2)]

        with tc.tile_pool(name="psmlp", bufs=3, space="PSUM") as psmlp, \
             tc.tile_pool(name="o_pool", bufs=2) as o_pool:
            fps = {}

            def fc1_hs(m, hi):
                hs = _halves()[hi]
                for k2 in range(KP):
                    nc.tensor.matmul(fps[m][:, hs],
                                     wf18[:, k2, :, m * P:(m + 1) * P],
                                     nx2p[k2][:, :, hs],
                                     start=(k2 == 0), stop=(k2 == KP - 1),
                                     perf_mode=DR)

            def fc1_gelu(m):
                nc.scalar.activation(h8[m // 2][:, m % 2, :], fps[m][:],
                                     AF.Gelu, bias=f1b[:, m:m + 1],
                                     scale=1.0 / 16.0)

            for m in range(MFC1):
                fps[m] = psmlp.tile([P, NT], F32, tag="mlp", name=f"fc1ps{m}")
                fc1_hs(m, 0)
                if m >= 2:
                    fc1_hs(m - 2, 1)
                    fc1_gelu(m - 2)
            for m in (MFC1 - 2, MFC1 - 1):
                fc1_hs(m, 1)
                fc1_gelu(m)

            # fc2 + residual
            for m in range(KC):
                ps = psmlp.tile([P, NT], F32, tag="mlp", name=f"fc2ps{m}")
                for hi, hs in enumerate(_halves()):
                    for m2 in range(MP2):
                        nc.tensor.matmul(ps[:, hs],
                                         wf28[:, m2, :, m * P:(m + 1) * P],
                                         h8[m2][:, :, hs],
                                         start=(m2 == 0), stop=False,
                                         perf_mode=DR)
                    # accumulate 16*fc2_b via a K=1 rank-1 matmul
                    nc.tensor.matmul(ps[:, hs],
                                     brow[0:1, D + m * P:D + (m + 1) * P],
                                     ones_row[0:1, hs], start=False, stop=True)
                ok = o_pool.tile([P, NT], F32, tag="o")
                # out = fc2_acc/16 + x1
                nc.vector.scalar_tensor_tensor(
                    ok[:], ps[:], 1.0 / 16.0, x1m[m][:].bitcast(F32),
                    ALU.mult, ALU.add)
                nc.sync.dma_start(outT[m * P:(m + 1) * P, :], ok[:])

    nc.compile()
    return nc


_NC = None


def _get_nc():
    global _NC
    if _NC is None:
        _NC = build()
    return _NC


def _pair_rows(wT):
    """[K, F] fp32 row-major -> [K//2, 2*F] fp8 with 128-row slab pairs:
    out[p*128+i, s*F+m] = wT[256*p + 128*s + i, m]."""
    K, F = wT.shape
    return np.ascontiguousarray(
        wT.reshape(K // 256, 2, 128, F).transpose(0, 2, 1, 3).reshape(K // 2, 2 * F)
    ).astype(ml_dtypes.float8_e4m3)


def _seg_of(p):
    return 0 if p < S1 else (1 if p < S2 else 2)


def _prep_inputs(inputs):
    f32 = np.float32
    bf16 = ml_dtypes.bfloat16
    g = {k: np.asarray(v) for k, v in inputs.items()}
    qkvT = np.ascontiguousarray(g["qkv_w"].astype(f32).T)  # [D, 3D]

    g1 = np.concatenate([g["ln1a_g"], g["ln1b_g"], g["ln1c_g"]]).astype(f32)
    b1 = np.concatenate([g["ln1a_b"], g["ln1b_b"], g["ln1c_b"]]).astype(f32)
    g2 = np.concatenate([g["ln2a_g"], g["ln2b_g"], g["ln2c_g"]]).astype(f32)
    b2 = np.concatenate([g["ln2a_b"], g["ln2b_b"], g["ln2c_b"]]).astype(f32)

    cpack = np.zeros((P, 49), dtype=f32)
    cpack[:, 0:6] = g["proj_b"].astype(f32).reshape(6, P).T
    cpack[:, 6:30] = g["fc1_b"].astype(f32).reshape(24, P).T
    cpack[:, 30:36] = g["fc2_b"].astype(f32).reshape(6, P).T
    cpack[:, 36:42] = b1.reshape(6, P).T
    cpack[:, 42:48] = b2.reshape(6, P).T
    cpack[0:3, 48] = [1.0 / S1, 1.0 / (S2 - S1), 1.0 / (D - S2)]

    def sg_pack(gv, bv):
        sg = np.zeros((4, D), dtype=f32)
        for p in range(D):
            sg[_seg_of(p), p] = gv[p]
        sg[3, :] = bv
        return sg

    inds = np.zeros((P, 3 * KC), dtype=f32)
    for k in range(KC):
        for p in range(P):
            inds[p, 3 * k + _seg_of(k * P + p)] = 1.0

    shared = {
        "wqk8": _pair_rows(16.0 * qkvT[:, 0:2 * D]),
        "wv8": _pair_rows(16.0 * qkvT[:, 2 * D:3 * D]),
        "wp8": _pair_rows(16.0 * np.ascontiguousarray(g["proj_w"].astype(f32).T)),
        "wf18": _pair_rows(16.0 * np.ascontiguousarray(g["fc1_w"].astype(f32).T)),
        "wf28": _pair_rows(16.0 * np.ascontiguousarray(g["fc2_w"].astype(f32).T)),
        "cpack": cpack,
        "brow": np.concatenate([256.0 * g["proj_b"].astype(f32),
                                16.0 * g["fc2_b"].astype(f32)]
                               ).reshape(1, 2 * D).astype(bf16),
        "sg1": sg_pack(g1, b1),
        "sg2": sg_pack(g2, b2),
        "indsum": inds,
    }
    x = np.asarray(g["x"], dtype=f32)
    in_maps = []
    for b in range(B):
        m = dict(shared)
        m["xT"] = np.ascontiguousarray(x[b].T)
        in_maps.append(m)
    return in_maps


def run(inputs, trace=False):
    nc = _get_nc()
    in_maps = _prep_inputs(inputs)
    res = run_bass_kernel_spmd(nc, in_maps, core_ids=list(range(B)),
                               trace=trace)
    out = np.stack([np.ascontiguousarray(res.results[b]["outT"].T)
                    for b in range(B)]).astype(np.float32)
    return out, res


def kernel(**inputs):
    out, _ = run(inputs, trace=False)
    return out


# revision 31
# speedup vs baseline: 1.0478x; 1.0478x over previous
"""Trainium2 Bass kernel for a dense transformer block.

Block: split-LayerNorm -> attention -> residual -> split-LayerNorm -> MLP(GELU)
-> residual.  Shapes: B=8, N=1024, D=768, H=12 heads (hd=64), HID=3072.

Sharding: pure data-parallel over batch -- one batch element per NeuronCore
(8 cores), all weights replicated, no collectives.

On-chip layout is feature-major (activations stored transposed, [feature,
token]).  The heavy GEMMs (qkv, attn*V, proj, fc1, fc2) run in fp8e4m3 with
MatmulPerfMode.DoubleRow (K=256 contraction per pass).  LayerNorm statistics
use float32r moving operands against tiny [128,3] indicator stationaries; the
per-token scale/offset (gamma*rstd, -gamma*mean*rstd + beta) are broadcast to
all 128 feature partitions with one [4,128]x[4,512] matmul each, so the apply
step is two plain tensor_tensor ops writing fp8 directly.  rstd is computed as
exp(-0.5*ln(var+eps)) so the scalar engine stays on the natural_log_exp table
set through LayerNorm and softmax (no sqrt-table swaps).  Attention scores use
K=64 partition-offset matmuls directly on the stacked q/k tiles (no zero-padded
key copies).
"""

import os
import numpy as np
import ml_dtypes

import concourse.bass as bass
import concourse.tile as tile
from concourse import bacc, mybir
from concourse.bass_utils import run_bass_kernel_spmd
from contextlib import ExitStack

F32 = mybir.dt.float32
F32R = mybir.dt.float32r
BF16 = mybir.dt.bfloat16
F8 = mybir.dt.float8e4
AF = mybir.ActivationFunctionType
ALU = mybir.AluOpType
DR = mybir.MatmulPerfMode.DoubleRow

D = 768
H = 12
HD = 64
HID = 3072
NT = 1024  # tokens per core
B = 8
S1 = 320  # split-LN segment boundaries: [0,320), [320,384), [384,768)
S2 = 384
SCALE = 0.125  # (D//H) ** -0.5 = 64 ** -0.5
EPS = 1e-5
P = 128

KC = D // P        # 6  c-chunks
KP = KC // 2       # 3  c-chunk pairs
MFC1 = HID // P    # 24
MP2 = MFC1 // 2    # 12 fc2 contraction pairs
VG = 80            # per-head col group in vT tiles: 64 dims + 1 ones + pad


def _halves():
    return (slice(0, 512), slice(512, 1024))


def _layernorm(tc, xat, sg4, bcols, indsum_s, invlen, eps_t, out_slab, tag,
               sq_scalar=False):
    """Split-LayerNorm over the feature dim (partitions), full token width.
    xat(k) returns the [128, NT] f32r feature-major input tile for chunk k.
    out_slab(k) returns the [128, NT] fp8 output AP for chunk k.
    sg4 is [4, D] f32r: rows 0..2 = gamma[p] masked by segment, row 3 =
    beta[p].  The scale/offset broadcast matmuls consume mv rows 0..2
    (rstd / -mean*rstd) plus a constant row 3 (0 / 1) so beta rides along."""
    nc = tc.nc
    with tc.tile_pool(name=f"lnsq_{tag}", bufs=3) as sqp, \
         tc.tile_pool(name=f"lnt_{tag}", bufs=3) as tp, \
         tc.tile_pool(name=f"lns_{tag}", bufs=1) as lns, \
         tc.tile_pool(name=f"lnps_{tag}", bufs=2, space="PSUM") as psstat, \
         tc.tile_pool(name=f"lnpb_{tag}", bufs=2, space="PSUM") as psb:
        sums = psstat.tile([3, NT], F32, tag="stat", name=f"sums_{tag}")
        sumsq = psstat.tile([3, NT], F32, tag="stat", name=f"sumsq_{tag}")
        xqs = [sqp.tile([P, NT], F32R, tag="xq", name=f"xq_{tag}_{k}")
               for k in range(KC)]
        for hi in range(2):
            hs = _halves()[hi]
            for k in range(KC):
                xk = xat(k)
                xf = xk[:, hs].bitcast(F32)
                if sq_scalar:
                    nc.scalar.activation(xqs[k][:, hs], xf, AF.Square)
                else:
                    nc.vector.tensor_tensor(xqs[k][:, hs], xf, xf, ALU.mult)
                nc.tensor.matmul(sums[:, hs], indsum_s[:, 3 * k:3 * k + 3],
                                 xk[:, hs],
                                 start=(k == 0), stop=(k == KC - 1))
                nc.tensor.matmul(sumsq[:, hs], indsum_s[:, 3 * k:3 * k + 3],
                                 xqs[k][:, hs],
                                 start=(k == 0), stop=(k == KC - 1))
        mean = lns.tile([3, NT], F32, tag="mean")
        nmsq = lns.tile([3, NT], F32, tag="nmsq")
        var = lns.tile([3, NT], F32, tag="var")
        std = lns.tile([3, NT], F32, tag="std")
        rcp = lns.tile([3, NT], F32, tag="rcp")
        # mv[:,0,:] = rstd ; mv[:,1,:] = -mean*rstd
        mv = lns.tile([3, 2, NT], F32R, tag="mv")
        nc.vector.tensor_scalar_mul(mean[:], sums[:], invlen[:])
        nc.vector.scalar_tensor_tensor(nmsq[:], mean[:], -1.0,
                                       mean[:], ALU.mult, ALU.mult)
        nc.vector.scalar_tensor_tensor(var[:], sumsq[:], invlen[:],
                                       nmsq[:], ALU.mult, ALU.add)
        nc.scalar.activation(std[:], var[:], AF.Sqrt, bias=eps_t[:])
        nc.vector.reciprocal_approx_fast(rcp[:], std[:])
        nc.vector.tensor_copy(mv[0:3, 0, :], rcp[:])
        nc.vector.scalar_tensor_tensor(mv[0:3, 1, :], mean[:], -1.0,
                                       rcp[:], ALU.mult, ALU.mult)
        for k in range(KC):
            sg_k = sg4[0:3, k * P:(k + 1) * P]
            scaleB = psb.tile([P, NT], F32, tag="bb")
            offB = psb.tile([P, NT], F32, tag="bb")
            for hs in _halves():
                nc.tensor.matmul(scaleB[:, hs], sg_k, mv[:, 0, hs],
                                 start=True, stop=True)
                nc.tensor.matmul(offB[:, hs], sg_k, mv[:, 1, hs],
                                 start=True, stop=True)
            tmp = tp.tile([P, NT], F32, tag="tmp", name=f"tmp_{tag}_{k}")
            nc.vector.tensor_tensor(tmp[:], xat(k).bitcast(F32), scaleB[:],
                                    ALU.mult)
            # out = (g*(-mean*rstd)_bcast + beta) + g*rstd*x
            nc.vector.scalar_tensor_tensor(out_slab(k), offB[:],
                                           bcols[:, k:k + 1], tmp[:],
                                           ALU.add, ALU.add)


def build():
    nc = bacc.Bacc("TRN2", target_bir_lowering=False, debug=False)

    xT = nc.dram_tensor("xT", [D, NT], F32R, kind="ExternalInput")
    wqk8d = nc.dram_tensor("wqk8", [KP * P, 2 * 2 * D], F8, kind="ExternalInput")
    wv8d = nc.dram_tensor("wv8", [KP * P, 2 * D], F8, kind="ExternalInput")
    wp8d = nc.dram_tensor("wp8", [KP * P, 2 * D], F8, kind="ExternalInput")
    wf18d = nc.dram_tensor("wf18", [KP * P, 2 * HID], F8, kind="ExternalInput")
    wf28d = nc.dram_tensor("wf28", [MP2 * P, 2 * D], F8, kind="ExternalInput")
    # constpack cols: 0:6 proj_b, 6:30 fc1_b, 30:36 fc2_b, 36:42 ln1_b,
    # 42:48 ln2_b, 48 invlen (rows 0..2)
    cpackd = nc.dram_tensor("cpack", [P, 49], F32, kind="ExternalInput")
    browd = nc.dram_tensor("brow", [1, 2 * D], BF16, kind="ExternalInput")
    sg1d = nc.dram_tensor("sg1", [4, D], F32R, kind="ExternalInput")
    sg2d = nc.dram_tensor("sg2", [4, D], F32R, kind="ExternalInput")
    indsd = nc.dram_tensor("indsum", [P, 3 * KC], F32R, kind="ExternalInput")
    outT = nc.dram_tensor("outT", [D, NT], F32, kind="ExternalOutput")

    with tile.TileContext(nc) as tc, ExitStack() as ctx:
        const = ctx.enter_context(tc.tile_pool(name="const", bufs=1))

        # constants
        eps_t = const.tile([3, 1], F32)
        nc.vector.memset(eps_t[:], EPS)
        # prewarm activation tables while the x DMA streams in.  Ln first so
        # the natural_log_exp set (which contains Exp too) is resident.
        warm = const.tile([1, 1], F32)
        nc.vector.memset(warm[:], 0.25)
        for fn in (AF.Exp, AF.Sqrt):
            wo = const.tile([1, 1], F32, tag=f"warm_{fn}")
            nc.scalar.activation(wo[:], warm[:], fn)

        # x first: LN1 start gates everything.  Column-split so the first
        # token-half lands before the second.
        x0pool = ctx.enter_context(tc.tile_pool(name="x0pool", bufs=KC))
        x_tiles = [x0pool.tile([P, NT], F32R, tag="x0", name=f"x0_{k}")
                   for k in range(KC)]
        for hs in _halves():
            for k in range(KC):
                nc.sync.dma_start(x_tiles[k][:, hs],
                                  xT[k * P:(k + 1) * P, hs])

        # small consts + attention weights on the scalar (Act) HWDGE queue so
        # they don't serialize behind x on the sync queue.
        indsum_s = const.tile([P, 3 * KC], F32R)
        nc.scalar.dma_start(indsum_s[:], indsd[:])
        sg1 = const.tile([4, D], F32R)
        nc.scalar.dma_start(sg1[:], sg1d[:])
        sg2 = const.tile([4, D], F32R)
        nc.scalar.dma_start(sg2[:], sg2d[:])
        cpack = const.tile([P, 49], F32)
        nc.scalar.dma_start(cpack[:], cpackd[:])
        pb = cpack[:, 0:6]
        f1b = cpack[:, 6:30]
        f2b = cpack[:, 30:36]
        b1c = cpack[:, 36:42]
        b2c = cpack[:, 42:48]
        invlen = cpack[0:3, 48:49]
        ones_row = const.tile([1, NT], BF16)
        nc.vector.memset(ones_row[:], 1.0)
        brow = const.tile([1, 2 * D], BF16)
        nc.scalar.dma_start(brow[:], browd[:])

        # proj weight pool first (outlives the attention-weight pool); its DMA
        # is issued after the attention weights so they win the bandwidth race.
        wlate = ctx.enter_context(ExitStack())
        wp_pool = wlate.enter_context(tc.tile_pool(name="wp_pool", bufs=1))
        wp8 = wp_pool.tile([P, KP, 2, D], F8, tag="wp8")
        # attention weights (single strided DMA each)
        wq_stage = ctx.enter_context(ExitStack())
        wq_pool = wq_stage.enter_context(tc.tile_pool(name="wqk", bufs=2))
        wqk8 = wq_pool.tile([P, KP, 2, 2 * D], F8, tag="wqk8")
        nc.sync.dma_start(
            wqk8[:], wqk8d.ap().rearrange("(a p) (s x) -> p a s x", p=P, s=2))
        wv8 = wq_pool.tile([P, KP, 2, D], F8, tag="wv8")
        nc.sync.dma_start(
            wv8[:], wv8d.ap().rearrange("(a p) (s x) -> p a s x", p=P, s=2))
        nc.sync.dma_start(
            wp8[:], wp8d.ap().rearrange("(a p) (s x) -> p a s x", p=P, s=2))

        # ---- LN1 + qkv + vT, pipelined per token-half ----
        v_stage = ctx.enter_context(ExitStack())   # vT tiles, live thru attention
        v_pool = v_stage.enter_context(tc.tile_pool(name="v_pool", bufs=4))
        nx_stage = ctx.enter_context(ExitStack())  # normx pairs, live thru qkv/vT
        nx_pool = nx_stage.enter_context(tc.tile_pool(name="nx_pool", bufs=KP))
        nxp = [nx_pool.tile([P, 2, NT], F8, tag="nxp", name=f"nxp{i}")
               for i in range(KP)]
        y_stage = ctx.enter_context(ExitStack())
        y_pool = y_stage.enter_context(
            tc.tile_pool(name="y_pool", bufs=KP, side="right"))
        ytp = [y_pool.tile([P, 2, NT], F8, tag="ytp", name=f"ytp{i}")
               for i in range(KP)]
        qkv_stage = ctx.enter_context(ExitStack())
        q_pool = qkv_stage.enter_context(
            tc.tile_pool(name="q_pool", bufs=12, side="right"))
        qt = [q_pool.tile([P, NT], BF16, tag="qkt", name=f"qt{m}")
              for m in range(2 * KC)]
        vtp = []
        for p2 in range(4):
            vt = v_pool.tile([P, 2, H * VG], F8, tag="vtp", name=f"vtp{p2}")
            # ones columns (col 64 of each 80-wide head group) for denominators
            for s in range(2):
                nc.vector.memset(
                    vt[:, s, :].rearrange("p (h c) -> p h c", c=VG)[:, :, HD:VG],
                    1.0)
            vtp.append(vt)

        # ---- LN1 both halves, then a dense qkv+vT block, then attention.
        # Keeping the dense matmul stream contiguous locks the PE clock gate
        # at full rate before the (scalar-bound) softmax stream starts. ----
        _layernorm(tc, lambda k: x_tiles[k][:], sg1, b1c, indsum_s, invlen,
                   eps_t, lambda k: nxp[k // 2][:, k % 2, :], "ln1")

        psqv_stage = ctx.enter_context(ExitStack())
        psqv = psqv_stage.enter_context(
            tc.tile_pool(name="psqv", bufs=4, space="PSUM"))

        # q/k chunks in head-consumption order; PSUM->SBUF copies alternate
        # between the scalar and vector engines so neither becomes the drain.
        QKV_ORDER = [0, 6, 1, 7, 2, 8, 3, 9, 4, 10, 5, 11]
        nu = 0
        for m in QKV_ORDER:
            for hi in range(2):
                hs = _halves()[hi]
                ps = psqv.tile([P, NT], F32, tag="qv", name=f"qv_{hi}_{m}")
                for k2 in range(KP):
                    nc.tensor.matmul(ps[:, 0:512],
                                     wqk8[:, k2, :, m * P:(m + 1) * P],
                                     nxp[k2][:, :, hs],
                                     start=(k2 == 0), stop=(k2 == KP - 1),
                                     perf_mode=DR)
                if nu % 2 == 0:
                    nc.scalar.activation(qt[m][:, hs], ps[:, 0:512], AF.Identity)
                else:
                    nc.vector.tensor_copy(qt[m][:, hs], ps[:, 0:512])
                nu += 1
        for tc_i in range(8):
            vps = psqv.tile([P, NT], F32, tag="qv", name=f"vps_{tc_i}")
            for cs in (slice(0, 512), slice(512, D)):
                for k2 in range(KP):
                    nc.tensor.matmul(vps[:, cs],
                                     nxp[k2][:, :, tc_i * P:(tc_i + 1) * P],
                                     wv8[:, k2, :, cs],
                                     start=(k2 == 0), stop=(k2 == KP - 1),
                                     perf_mode=DR)
            dst = vtp[tc_i // 2][:, tc_i % 2, :].rearrange(
                "p (h c) -> p h c", c=VG)[:, :, 0:HD]
            nc.vector.tensor_copy(
                dst, vps[:, 0:D].rearrange("p (h c) -> p h c", c=HD))
        # pre-load the exp table while the qkv block runs
        wexp = const.tile([1, 1], F32, tag="warm_exp2")
        nc.scalar.activation(wexp[:], warm[:], AF.Exp)
        psqv_stage.close()

        # ---- attention: scores/exp stream with 1-head lagged attn*V and
        # the late q/k chunks interleaved as sparse fillers ----
        psmm_stage = ctx.enter_context(ExitStack())
        psmm = psmm_stage.enter_context(
            tc.tile_pool(name="psmm", bufs=2, space="PSUM"))
        with tc.tile_pool(name="e_pool", bufs=10) as e_pool, \
             tc.tile_pool(name="kp_pool", bufs=4) as kp_pool, \
             tc.tile_pool(name="psot", bufs=2, space="PSUM") as psot, \
             tc.tile_pool(name="sm_pool", bufs=2) as sm_pool:

            # alternating zero-padded key tiles: the zero half is set once
            # and never overwritten (head parity matches buffer parity)
            kp_bufs = [kp_pool.tile([P, NT], BF16, tag="kp", name=f"kp{i}")
                       for i in range(4)]
            for i in range(4):
                if i % 2 == 0:
                    nc.vector.memset(kp_bufs[i][HD:P, :], 0.0)
                else:
                    nc.vector.memset(kp_bufs[i][0:HD, :], 0.0)

            def make_tail(h, ot):
                def tail():
                    dnm = sm_pool.tile([1, NT], F32, tag="dnm")
                    nc.vector.tensor_copy(dnm[:], ot[HD:HD + 1, :])
                    r = sm_pool.tile([1, NT], F32, tag="recip")
                    nc.vector.reciprocal_approx_fast(r[:], dnm[:])
                    rbs = sm_pool.tile([HD, NT], F32, tag="rbs")
                    nc.gpsimd.partition_broadcast(rbs[:], r[:])
                    po2 = (h % 2) * HD
                    nc.vector.tensor_mul(
                        ytp[h // 4][po2:po2 + HD, (h // 2) % 2, :],
                        ot[0:HD, :], rbs[:])
                return tail

            av_queue = []
            for h in range(H):
                j = h // 2
                po = (h % 2) * HD
                qsl = qt[j]
                ksl = qt[KC + j]
                kp = kp_bufs[h % 4]
                nc.vector.tensor_copy(kp[po:po + HD, :], ksl[po:po + HD, :])
                ot = psot.tile([P, NT], F32, tag="ot")
                exs = [e_pool.tile([P, 2, NT], F8, tag="ex", name=f"ex{h}_{i}")
                       for i in range(4)]

                def av(p2, ot=ot, exs=exs, hh=h, tl=None):
                    for hs in _halves():
                        nc.tensor.matmul(
                            ot[0:HD + 2, hs],
                            vtp[p2][:, :, hh * VG:hh * VG + HD + 2],
                            exs[p2][:, :, hs],
                            start=(p2 == 0), stop=(p2 == 3),
                            perf_mode=DR)
                    if tl is not None:
                        tl()

                for kc in range(8):
                    st = psmm.tile([P, NT], F32, tag="mm")
                    for hs in _halves():
                        nc.tensor.matmul(
                            st[:, hs],
                            kp[:, kc * P:(kc + 1) * P],
                            qsl[:, hs],
                            start=True, stop=True)
                    nc.scalar.activation(exs[kc // 2][:, kc % 2, :],
                                         st[:], AF.Exp, scale=SCALE / 256.0)
                    if kc % 2 == 1 and av_queue:
                        av_queue.pop(0)()
                tl = make_tail(h, ot)
                av_queue += [
                    lambda av=av: av(0), lambda av=av: av(1),
                    lambda av=av: av(2), lambda av=av, tl=tl: av(3, tl=tl)]
            while av_queue:
                av_queue.pop(0)()
        nx_stage.close()
        v_stage.close()
        wq_stage.close()
        qkv_stage.close()
        psmm_stage.close()

        # MLP weights: issue DMAs now (consumed ~20us later)
        wf1_pool = wlate.enter_context(tc.tile_pool(name="wf1_pool", bufs=1))
        wf18 = wf1_pool.tile([P, KP, 2, HID], F8, tag="wf18")
        nc.sync.dma_start(
            wf18[:], wf18d.ap().rearrange("(a p) (s x) -> p a s x", p=P, s=2))
        wf2_pool = wlate.enter_context(tc.tile_pool(name="wf2_pool", bufs=1))
        wf28 = wf2_pool.tile([P, MP2, 2, D], F8, tag="wf28")
        nc.sync.dma_start(
            wf28[:], wf28d.ap().rearrange("(a p) (s x) -> p a s x", p=P, s=2))
        x1pool = ctx.enter_context(tc.tile_pool(name="x1pool", bufs=KC))
        x1m = [x1pool.tile([P, NT], F32R, tag="x1", name=f"x1_{m}")
               for m in range(KC)]
        nx2_stage = ctx.enter_context(ExitStack())
        nx2_pool = nx2_stage.enter_context(tc.tile_pool(name="nx2_pool", bufs=KP))
        nx2p = [nx2_pool.tile([P, 2, NT], F8, tag="nx2p", name=f"nx2p{i}")
                for i in range(KP)]

        # ---- proj + residual (full token width), then LN2 ----
        with tc.tile_pool(name="pspj", bufs=2, space="PSUM") as pspj, \
             tc.tile_pool(name="ptp", bufs=2) as ptp:
            for m in range(KC):
                ps = pspj.tile([P, NT], F32, tag="pj")
                for hi, hs in enumerate(_halves()):
                    for k2 in range(KP):
                        nc.tensor.matmul(ps[:, hs],
                                         wp8[:, k2, :, m * P:(m + 1) * P],
                                         ytp[k2][:, :, hs],
                                         start=(k2 == 0), stop=(k2 == KP - 1),
                                         perf_mode=DR)
                pt = ptp.tile([P, NT], F32, tag="pt")
                nc.scalar.activation(pt[:], ps[:], AF.Identity,
                                     bias=pb[:, m:m + 1], scale=1.0 / 256.0)
                # x1 = proj_out + proj_b + x
                nc.vector.tensor_tensor(
                    x1m[m][:], pt[:], x_tiles[m][:].bitcast(F32), ALU.add)
        _layernorm(tc, lambda k: x1m[k][:], sg2, b2c, indsum_s, invlen,
                   eps_t, lambda k: nx2p[k // 2][:, k % 2, :], "ln2",
                   sq_scalar=True)
        y_stage.close()

        # ---- MLP, full token width, fc1 software-pipelined by 2 chunks ----
        h_stage = ctx.enter_context(ExitStack())
        h_pool = h_stage.enter_context(
            tc.tile_pool(name="h_pool", bufs=MP2, side="right"))
        h8 = [h_pool.tile([P, 2, NT], F8, tag="h8", name=f"h8_{i}")
              for i in range(MP2)]

        with tc.tile_pool(name="psmlp", bufs=3, space="PSUM") as psmlp, \
             tc.tile_pool(name="o_pool", bufs=2) as o_pool:
            fps = {}

            def fc1_hs(m, hi):
                hs = _halves()[hi]
                for k2 in range(KP):
                    nc.tensor.matmul(fps[m][:, hs],
                                     wf18[:, k2, :, m * P:(m + 1) * P],
                                     nx2p[k2][:, :, hs],
                                     start=(k2 == 0), stop=(k2 == KP - 1),
                                     perf_mode=DR)

            def fc1_gelu(m):
                nc.scalar.activation(h8[m // 2][:, m % 2, :], fps[m][:],
                                     AF.Gelu, bias=f1b[:, m:m + 1],
                                     scale=1.0 / 16.0)

            for m in range(MFC1):
                fps[m] = psmlp.tile([P, NT], F32, tag="mlp", name=f"fc1ps{m}")
                fc1_hs(m, 0)
                if m >= 2:
                    fc1_hs(m - 2, 1)
                    fc1_gelu(m - 2)
            for m in (MFC1 - 2, MFC1 - 1):
                fc1_hs(m, 1)
                fc1_gelu(m)

            # fc2 + residual
            for m in range(KC):
                ps = psmlp.tile([P, NT], F32, tag="mlp", name=f"fc2ps{m}")
                for hi, hs in enumerate(_halves()):
                    for m2 in range(MP2):
                        nc.tensor.matmul(ps[:, hs],
                                         wf28[:, m2, :, m * P:(m + 1) * P],
                                         h8[m2][:, :, hs],
                                         start=(m2 == 0), stop=(m2 == MP2 - 1),
                                         perf_mode=DR)
                ft = o_pool.tile([P, NT], F32, tag="ft")
                nc.scalar.activation(ft[:], ps[:], AF.Identity,
                                     bias=f2b[:, m:m + 1], scale=1.0 / 16.0)
                ok = o_pool.tile([P, NT], F32, tag="o")
                # out = fc2_out + fc2_b + x1
                nc.vector.tensor_tensor(
                    ok[:], ft[:], x1m[m][:].bitcast(F32), ALU.add)
                nc.sync.dma_start(outT[m * P:(m + 1) * P, :], ok[:])

    nc.compile()
    return nc


_NC = None


def _get_nc():
    global _NC
    if _NC is None:
        _NC = build()
    return _NC


def _pair_rows(wT):
    """[K, F] fp32 row-major -> [K//2, 2*F] fp8 with 128-row slab pairs:
    out[p*128+i, s*F+m] = wT[256*p + 128*s + i, m]."""
    K, F = wT.shape
    return np.ascontiguousarray(
        wT.reshape(K // 256, 2, 128, F).transpose(0, 2, 1, 3).reshape(K // 2, 2 * F)
    ).astype(ml_dtypes.float8_e4m3)


def _seg_of(p):
    return 0 if p < S1 else (1 if p < S2 else 2)


def _prep_inputs(inputs):
    f32 = np.float32
    bf16 = ml_dtypes.bfloat16
    g = {k: np.asarray(v) for k, v in inputs.items()}
    qkvT = np.ascontiguousarray(g["qkv_w"].astype(f32).T)  # [D, 3D]

    g1 = np.concatenate([g["ln1a_g"], g["ln1b_g"], g["ln1c_g"]]).astype(f32)
    b1 = np.concatenate([g["ln1a_b"], g["ln1b_b"], g["ln1c_b"]]).astype(f32)
    g2 = np.concatenate([g["ln2a_g"], g["ln2b_g"], g["ln2c_g"]]).astype(f32)
    b2 = np.concatenate([g["ln2a_b"], g["ln2b_b"], g["ln2c_b"]]).astype(f32)

    cpack = np.zeros((P, 49), dtype=f32)
    cpack[:, 0:6] = g["proj_b"].astype(f32).reshape(6, P).T
    cpack[:, 6:30] = g["fc1_b"].astype(f32).reshape(24, P).T
    cpack[:, 30:36] = g["fc2_b"].astype(f32).reshape(6, P).T
    cpack[:, 36:42] = b1.reshape(6, P).T
    cpack[:, 42:48] = b2.reshape(6, P).T
    cpack[0:3, 48] = [1.0 / S1, 1.0 / (S2 - S1), 1.0 / (D - S2)]

    def sg_pack(gv, bv):
        sg = np.zeros((4, D), dtype=f32)
        for p in range(D):
            sg[_seg_of(p), p] = gv[p]
        sg[3, :] = bv
        return sg

    inds = np.zeros((P, 3 * KC), dtype=f32)
    for k in range(KC):
        for p in range(P):
            inds[p, 3 * k + _seg_of(k * P + p)] = 1.0

    shared = {
        "wqk8": _pair_rows(16.0 * qkvT[:, 0:2 * D]),
        "wv8": _pair_rows(16.0 * qkvT[:, 2 * D:3 * D]),
        "wp8": _pair_rows(16.0 * np.ascontiguousarray(g["proj_w"].astype(f32).T)),
        "wf18": _pair_rows(16.0 * np.ascontiguousarray(g["fc1_w"].astype(f32).T)),
        "wf28": _pair_rows(16.0 * np.ascontiguousarray(g["fc2_w"].astype(f32).T)),
        "cpack": cpack,
        "brow": np.concatenate([256.0 * g["proj_b"].astype(f32),
                                16.0 * g["fc2_b"].astype(f32)]
                               ).reshape(1, 2 * D).astype(bf16),
        "sg1": sg_pack(g1, b1),
        "sg2": sg_pack(g2, b2),
        "indsum": inds,
    }
    x = np.asarray(g["x"], dtype=f32)
    in_maps = []
    for b in range(B):
        m = dict(shared)
        m["xT"] = np.ascontiguousarray(x[b].T)
        in_maps.append(m)
    return in_maps


def run(inputs, trace=False):
    nc = _get_nc()
    in_maps = _prep_inputs(inputs)
    res = run_bass_kernel_spmd(nc, in_maps, core_ids=list(range(B)),
                               trace=trace)
    out = np.stack([np.ascontiguousarray(res.results[b]["outT"].T)
                    for b in range(B)]).astype(np.float32)
    return out, res


def kernel(**inputs):
    out, _ = run(inputs, trace=False)
    return out
